# revision 46
# baseline (speedup 1.0000x reference)
"""Single-head causal attention (B=8, T=2048, D=1024, fp32 I/O) on 8 trn2
NeuronCores, data-parallel over batch (one batch element per core).

Per-core algorithm (matmuls bf16 except scores, with fp32 PSUM accum):
  xT   = transpose(cast_bf16(x))   t0-7: PE-transpose; t8-15: DRAM+xbar
  q8T  = Wq^T-stationary matmuls  -> (e, t) layout, evacuated to fp8-e4m3
  k8T  = same                                     -> (e, t) fp8
  v    = xT-stationary matmuls    -> (t, e) layout, with a ones column
         appended at e=1024 so the AV matmul also produces the rowsums
  S^T  block (j, i) = k8T(:,j)-stationary @ q8T   (fp8 DoubleRow matmuls:
         256-row contraction per instruction = 2x bf16 throughput)
  E^T  = exp(S^T / 32)   (no max-subtraction needed: |S/32| <~ 2)
         diagonal blocks masked by an upper-triangular 0/1 multiply
  out  = (E^T-stationary @ v_ext) * (1/rowsum)    per-partition scalar scale
         (AV split in 3 psum banks: cols 0:384, 384:768, 768:1025; the
         last carries the appended-ones rowsum column at psum col 256)

The softmax normalization is applied to the AV output instead of to the
weights, so no transposes of the (T, T) attention matrix are ever needed.
Score spans are aligned to the causal diagonal so no masked block is ever
computed except the triangular diagonal blocks themselves.  fp8 on the
score matmul only: quantizing q/k to e4m3 costs ~1.2e-2 rel-l2 on the
final output (vs the 2e-2 gate); fp8 on AV or QKV would exceed the gate.
"""
import sys
import types

import numpy as np

import concourse.bass as bass
import concourse.mybir as mybir
import concourse.tile as tile
from concourse.bass_utils import run_bass_kernel_spmd
from concourse.masks import make_identity, make_upper_triangular

B, T, D = 8, 2048, 1024
P = 128
TB = T // P        # 16 t-blocks
DBLK = D // P      # 8 d/e-blocks
NTS = T // 512     # 4 t-spans of 512
NES = D // 512     # 2 e-spans of 512
SCALE = 1.0 / 32.0  # 1/sqrt(D)

F32 = mybir.dt.float32
BF16 = mybir.dt.bfloat16
F8 = mybir.dt.float8e4
DR = mybir.MatmulPerfMode.DoubleRow
VW = 1032  # vsb row width: 1024 v cols + ones col at 1024 + 7 pad


def _install_ntff_hook():
    """Optional: register the axon NTFF profiling hook (the agent image's
    antenv lacks axon_hooks). Lets BASS_TRACE=1 produce exec_time_ns."""
    try:
        import antenv

        if "antenv.axon_hooks" in sys.modules:
            return
        mod = types.ModuleType("antenv.axon_hooks")
        _hook = [None]
        mod.set_axon_ntff_profile_hook = lambda h: _hook.__setitem__(0, h)
        mod.get_axon_ntff_profile_hook = lambda: _hook[0]
        sys.modules["antenv.axon_hooks"] = mod
        antenv.axon_hooks = mod
        from trn_agent_boot.trn_boot import _ntff_profile_via_ctypes

        mod.set_axon_ntff_profile_hook(
            _ntff_profile_via_ctypes("/opt/axon/libaxon_pjrt.so")
        )
    except Exception:
        pass


_install_ntff_hook()


def _split_multi_waits(nc: bass.Bass):
    """Walrus on this stack fits only ONE sync-wait per instruction, but
    Tile emits several on multi-producer instructions. Hoist the extra waits
    onto single-wait NoOps placed just before, on the same engine — the
    per-engine streams are in-order, so semantics are identical."""
    n_split = 0
    for fn in nc.m.functions:
        for bb in fn.blocks:
            out = []
            changed = False
            for inst in bb.instructions:
                si = inst.sync_info
                waits = list(si.on_wait) if si is not None and si.on_wait else []
                if len(waits) > 1:
                    for w in waits[:-1]:
                        nop = mybir.InstNoOp(
                            name=nc.get_next_instruction_name(),
                            engine=inst.engine,
                            ins=[],
                            outs=[],
                            sync_info=mybir.SyncInfo(on_wait=[w], on_update=[]),
                            bass_nofuse=True,
                        )
                        out.append(nop)
                    inst.sync_info = mybir.SyncInfo(
                        on_wait=[waits[-1]],
                        on_update=list(si.on_update or []),
                    )
                    changed = True
                    n_split += 1
                out.append(inst)
            if changed:
                bb.instructions = out
    return n_split


def _emit(nc: bass.Bass):
    x = nc.dram_tensor("x", [T, D], F32, kind="ExternalInput").ap()
    Wq = nc.dram_tensor("Wq", [D, D], F32, kind="ExternalInput").ap()
    Wk = nc.dram_tensor("Wk", [D, D], F32, kind="ExternalInput").ap()
    Wv = nc.dram_tensor("Wv", [D, D], F32, kind="ExternalInput").ap()
    out = nc.dram_tensor("out", [T, D], F32, kind="ExternalOutput").ap()

    with tile.TileContext(nc) as tc:
        from contextlib import ExitStack

        with ExitStack() as ctx:
            persist = ctx.enter_context(tc.tile_pool(name="persist", bufs=1))
            psum = ctx.enter_context(tc.tile_pool(name="psum", bufs=8, space="PSUM"))

            # ---- persistent SBUF tensors (survive the whole kernel) ----
            qT = persist.tile([P, DBLK, T], F8)         # (e, t) fp8
            kT = persist.tile([P, DBLK, T], F8)         # (e, t) fp8
            vsb = persist.tile([P, TB, VW], BF16)       # (t, e) + ones col
            # E^T tiles for i-spans 0-1 (computed early, inside phase B,
            # to fill the PE bubble while ts2/ts3 xT transposes land)
            etE = persist.tile([P, 12, 512], BF16)      # (j, i) blocks
            triu = persist.tile([P, P], BF16)
            ident = persist.tile([P, P], BF16)

            # identity first: it gates the very first PE transpose; the
            # masks are not needed until the attention phase.
            make_identity(nc, ident)
            # ones column of v_ext: the AV matmul rowsum trick.
            nc.gpsimd.memset(vsb[:, :, 1024:1025], 1.0)
            # triu[j, i] = 1 where i >= j (keep), 0 below the diagonal.
            make_upper_triangular(nc, triu, val=1.0, diag=True)

            # ======== Phase A+B: load/cast/transpose + A/h/v ===========
            # q/k projections are replaced by the associativity trick:
            #   S = q k^T = x (Wq Wk^T) x^T = h x^T   with  h = x A.
            # A = Wq Wk^T costs D^3 MACs where the k projection cost
            # T D^2 = 2 D^3: one whole projection is eliminated for the
            # price of A plus two weight transposes.
            dram = ctx.enter_context(tc.tile_pool(name="dram", bufs=1, space="DRAM"))
            with tc.tile_pool(name="qkvp", bufs=1) as qkvp, \
                 tc.tile_pool(name="staging", bufs=4) as staging:
                xT = qkvp.tile([P, DBLK, T], BF16)          # (d, t)
                WqT = qkvp.tile([P, DBLK, D], BF16)         # (e, d)
                WkT = qkvp.tile([P, DBLK, D], BF16)         # (e, d)
                A_sb = qkvp.tile([P, DBLK, D], BF16)        # (d, d')
                wv_bf = qkvp.tile([P, DBLK, D], BF16)

                xbf_dram = dram.tile([T, D], BF16)

                def x_chain_pe(tb):
                    """Load x row-block tb, cast to bf16, PE-transpose the
                    8 [128,128] sub-blocks into xT (fast path for the first
                    t-span, before the xbar pipeline has warmed up)."""
                    stage_f32 = staging.tile([P, D], F32, tag="xs32", bufs=3)
                    if tb < 2:
                        # split across two DMA queues: earlier first-byte for
                        # the tiles that gate the very first PE transposes
                        nc.sync.dma_start(
                            out=stage_f32[:, 0:512],
                            in_=x[tb * P:(tb + 1) * P, 0:512],
                        )
                        nc.sync.dma_start(
                            out=stage_f32[:, 512:D],
                            in_=x[tb * P:(tb + 1) * P, 512:D],
                        )
                    else:
                        nc.sync.dma_start(
                            out=stage_f32, in_=x[tb * P:(tb + 1) * P, :]
                        )
                    stage_bf = staging.tile([P, D], BF16, tag="xsbf", bufs=3)
                    # Strict engine ownership decouples the x and W DMA
                    # pipelines: x casts on DVE, W casts on ACT — an
                    # in-order queue never holds one stream's cast behind
                    # the other stream's DMA wait.
                    nc.vector.tensor_copy(stage_bf[:, 0:512], stage_f32[:, 0:512])
                    nc.vector.tensor_copy(stage_bf[:, 512:D], stage_f32[:, 512:D])
                    for dq in range(2):
                        # transpose as a REGULAR matmul (x-block stationary,
                        # identity moving): out = x_blk.T @ I. Issues ~2.5x
                        # faster than transpose_mode and warms HAM. 4 block
                        # transposes share one psum bank so the PE never
                        # waits on a per-block evacuation round-trip.
                        pst = psum.tile([P, 512], F32, tag="big")
                        for dd in range(4):
                            db = 4 * dq + dd
                            nc.tensor.matmul(
                                pst[:, dd * P:(dd + 1) * P],
                                lhsT=stage_bf[:, db * P:(db + 1) * P],
                                rhs=ident,
                                start=True,
                                stop=True,
                            )
                        for dd in range(4):
                            db = 4 * dq + dd
                            nc.vector.tensor_copy(
                                xT[:, db, tb * P:(tb + 1) * P],
                                pst[:, dd * P:(dd + 1) * P],
                            )

                def x_chain_store(tb):
                    """Load x row-block tb, cast to bf16, store to DRAM for
                    the xbar-transposed reload (keeps the PE free)."""
                    stage_f32 = staging.tile([P, D], F32, tag="xs32", bufs=3)
                    nc.sync.dma_start(
                        out=stage_f32, in_=x[tb * P:(tb + 1) * P, :]
                    )
                    stage_bf = staging.tile([P, D], BF16, tag="xsbf", bufs=3)
                    nc.vector.tensor_copy(stage_bf, stage_f32)
                    nc.sync.dma_start(
                        out=xbf_dram[tb * P:(tb + 1) * P, :], in_=stage_bf
                    )

                def xbar_batch(ts):
                    for db in range(DBLK):
                        nc.sync.dma_start_transpose(
                            out=xT[:, db, ts * 512:(ts + 1) * 512],
                            in_=xbf_dram[ts * 512:(ts + 1) * 512,
                                         db * P:(db + 1) * P],
                        )

                wcast = [0]
                evac_rr = [0]

                def evac(dst, src):
                    """Alternate PSUM evacuations between DVE and ACT (the
                    only engines with PSUM access) so neither backlog
                    stalls the PE's psum rotation."""
                    evac_rr[0] += 1
                    if evac_rr[0] % 2 == 0:
                        nc.vector.tensor_copy(dst, src)
                    else:
                        nc.scalar.copy(dst, src)

                def w_chain_half(w_dram, w_sb, db, half):
                    """Load half a W row-block (so the first v groups only
                    wait on 2MB of Wv, not all 4MB)."""
                    stage_f32 = staging.tile([P, 512], F32, tag="wvs32", bufs=2)
                    nc.gpsimd.dma_start(
                        out=stage_f32,
                        in_=w_dram[db * P:(db + 1) * P,
                                   half * 512:(half + 1) * 512],
                    )
                    nc.scalar.copy(
                        w_sb[:, db, half * 512:(half + 1) * 512], stage_f32
                    )

                def wT_chain(w_dram, wT_sb, db, alt_layout):
                    """Load W row-block db, cast bf16 into a rotating row
                    buffer, PE-transpose its 8 blocks.  alt_layout=True
                    stores block (eb, db) at wT_sb[:, db, eb*P:] (row-block
                    major: contiguous [P,512] evacuations; usable only for
                    per-block stationary slices).  False stores at
                    wT_sb[:, eb, db*P:] (e-major: spans of d' are sliceable
                    as a moving operand)."""
                    stage_f32 = staging.tile([P, D], F32, tag="ws32", bufs=2)
                    # W rides the Activation-hosted DMA queue set, disjoint
                    # from the sync-hosted queues that carry x: the two
                    # streams ramp in parallel instead of serializing.
                    nc.scalar.dma_start(
                        out=stage_f32, in_=w_dram[db * P:(db + 1) * P, :]
                    )
                    wrow = staging.tile([P, D], BF16, tag="wrow", bufs=2)
                    nc.scalar.copy(wrow, stage_f32)
                    for eq in range(2):
                        pst = psum.tile([P, 512], F32, tag="big")
                        for ee in range(4):
                            eb = 4 * eq + ee
                            nc.tensor.matmul(
                                pst[:, ee * P:(ee + 1) * P],
                                lhsT=wrow[:, eb * P:(eb + 1) * P],
                                rhs=ident,
                                start=True,
                                stop=True,
                            )
                        if alt_layout:
                            nc.vector.tensor_copy(
                                wT_sb[:, db, eq * 512:(eq + 1) * 512], pst
                            )
                        else:
                            for ee in range(4):
                                eb = 4 * eq + ee
                                nc.scalar.copy(
                                    wT_sb[:, eb, db * P:(db + 1) * P],
                                    pst[:, ee * P:(ee + 1) * P],
                                )

                def a_group(db, sp):
                    """A[d-block db, d'-span sp] = sum_e Wq[d,e] Wk[d',e].
                    WqT is in row-block-major (alt) layout, WkT in e-major
                    layout."""
                    ps = psum.tile([P, 512], F32, tag="big")
                    for eb in range(DBLK):
                        nc.tensor.matmul(
                            ps,
                            lhsT=WqT[:, db, eb * P:(eb + 1) * P],
                            rhs=WkT[:, eb, sp * 512:(sp + 1) * 512],
                            start=(eb == 0),
                            stop=(eb == DBLK - 1),
                        )
                    evac(A_sb[:, db, sp * 512:(sp + 1) * 512], ps)

                def h_group(dpb, ts):
                    """h^T[d'-block dpb, t-span ts] = sum_d A[d,d'] x^T[d,t],
                    evacuated straight to fp8 (h is only used by the fp8
                    score matmuls)."""
                    ps = psum.tile([P, 512], F32, tag="big")
                    for db in range(DBLK):
                        nc.tensor.matmul(
                            ps,
                            lhsT=A_sb[:, db, dpb * P:(dpb + 1) * P],
                            rhs=xT[:, db, ts * 512:(ts + 1) * 512],
                            start=(db == 0),
                            stop=(db == DBLK - 1),
                        )
                    evac(qT[:, dpb, ts * 512:(ts + 1) * 512], ps)

                def x8_cast(ts):
                    """fp8 copy of an xT t-span: the k-side score operand."""
                    for db in range(DBLK):
                        nc.any.tensor_copy(
                            kT[:, db, ts * 512:(ts + 1) * 512],
                            xT[:, db, ts * 512:(ts + 1) * 512],
                        )

                def score_exp(jb, i0, L, et_ap):
                    """S^T block row jb over i in [i0, i0+L): fp8 DoubleRow
                    matmuls (256-row contraction each), exp (scaled),
                    diagonal mask if the span starts on the causal
                    diagonal."""
                    ps = psum.tile([P, 512], F32, tag="big")
                    for p4 in range(DBLK // 2):
                        nc.tensor.matmul(
                            ps[:, 0:L],
                            lhsT=kT[:, 2 * p4:2 * p4 + 2, jb * P:(jb + 1) * P],
                            rhs=qT[:, 2 * p4:2 * p4 + 2, i0:i0 + L],
                            start=(p4 == 0),
                            stop=(p4 == DBLK // 2 - 1),
                            perf_mode=DR,
                        )
                    nc.scalar.activation(
                        et_ap, ps[:, 0:L],
                        mybir.ActivationFunctionType.Exp, scale=SCALE,
                    )
                    if jb * P >= i0:  # diagonal block leads this span
                        nc.vector.tensor_mul(
                            et_ap[:, 0:P], et_ap[:, 0:P], triu
                        )

                def v_group(tb, es):
                    """v[t(128), e(512)] = sum_d xT[d, t]-stat @ W[d, e]."""
                    ps = psum.tile([P, 512], F32, tag="big")
                    for db in range(DBLK):
                        nc.tensor.matmul(
                            ps,
                            lhsT=xT[:, db, tb * P:(tb + 1) * P],
                            rhs=wv_bf[:, db, es * 512:(es + 1) * 512],
                            start=(db == 0),
                            stop=(db == DBLK - 1),
                        )
                    evac(vsb[:, tb, es * 512:(es + 1) * 512], ps)

                # DMA delivery order matches PE consumption order: x tb0-7
                # and Wk's first half feed the first transposes, then Wq
                # row-blocks stream in for A span 0.  Wv halves load in
                # parallel on the gpsimd queue so v groups can interleave
                # with the A groups — the v matmuls depend on neither WqT
                # nor WkT, so they fill any transpose-evacuation stalls.
                # The PE p-state ramps down on any stall and needs ~3us of
                # continuous execution to reach full clock, so the schedule
                # runs LONG batches that start only once their inputs are
                # fully resident, instead of tracking DMA at fine grain:
                #   x transposes + v (x and Wv are the only early bytes)
                #   -> W transposes + A (Wq/Wk land during v)
                #   -> h -> early scores -> late v spans -> late h spans.
                # DMA order on the sync queues: x tb0-3, Wq, x tb4-7, Wk,
                # x-stores; Wv rides gpsimd queues in parallel.  PE batches
                # consume in the same order, each starting roughly when its
                # inputs have landed, with v spans as elastic filler.
                for db in range(DBLK):
                    w_chain_half(Wv, wv_bf, db, 0)   # gpsimd queues, t=0
                for i in range(4):
                    x_chain_pe(i)
                # One v group after each W-transpose row: the pair costs
                # ~2.1us of PE against ~1.4us of row DMA, so the PE runs
                # continuously at full p-state while tracking the W stream.
                for db in range(4):
                    wT_chain(Wq, WqT, db, alt_layout=True)
                    v_group(db, 0)
                for db in range(4, DBLK):
                    wT_chain(Wq, WqT, db, alt_layout=True)
                    x_chain_pe(db)
                    w_chain_half(Wv, wv_bf, 2 * (db - 4), 1)
                    w_chain_half(Wv, wv_bf, 2 * (db - 4) + 1, 1)
                for db in range(DBLK):
                    wT_chain(Wk, WkT, db, alt_layout=False)
                    if db % 2 == 0:
                        v_group(4 + db // 2, 0)
                    else:
                        v_group(4 + db // 2, 1)
                for tb in range(4):
                    v_group(tb, 1)
                for db in range(DBLK):
                    a_group(db, 0)
                    x_chain_store(8 + db)
                for db in range(DBLK):
                    a_group(db, 1)
                xbar_batch(2)
                for dpb in range(DBLK):
                    h_group(dpb, 0)
                x8_cast(0)
                x8_cast(1)
                xbar_batch(3)
                for dpb in range(DBLK):
                    h_group(dpb, 1)
                # Early scores for i-spans 0-1: PE work that is independent
                # of the xbar round-trip for x t-spans 2-3.
                eidx = 0
                for s in range(2):
                    for jb in range(4 * s + 4):
                        i0 = max(s * 512, jb * P)
                        L = (s + 1) * 512 - i0
                        score_exp(jb, i0, L, etE[:, eidx, 0:L])
                        eidx += 1
                for tb in range(8, 10):
                    for es in range(NES):
                        v_group(tb, es)
                for dpb in range(DBLK):
                    h_group(dpb, 2)
                x8_cast(2)
                for tb in range(10, 13):
                    for es in range(NES):
                        v_group(tb, es)
                for dpb in range(DBLK):
                    h_group(dpb, 3)
                x8_cast(3)
                for tb in range(13, TB):
                    for es in range(NES):
                        v_group(tb, es)

            # ================= Phase C+D: attention =====================
            with tc.tile_pool(name="etp", bufs=16) as etp, \
                 tc.tile_pool(name="outp", bufs=4) as outp, \
                 tc.tile_pool(name="rsp", bufs=4) as rsp:
                eidx = 0
                for s in range(NTS):
                    # --- scores + exp for i-span s, all jb <= 4s+3 ---
                    # (spans 0-1 were already computed inside phase B; see
                    # the early-scores fill)
                    et_tiles = []
                    et_i0 = []
                    for jb in range(4 * s + 4):
                        i0 = max(s * 512, jb * P)
                        L = (s + 1) * 512 - i0
                        if s < 2:
                            et = etE[:, eidx, :]
                            eidx += 1
                        else:
                            et = etp.tile([P, 512], BF16, tag="et")
                            score_exp(jb, i0, L, et[:, 0:L])
                        et_tiles.append(et)
                        et_i0.append(i0)

                    # --- AV (incl rowsum col) for the 4 i-blocks in span s ---
                    for ib in range(4 * s, 4 * s + 4):
                        ps0 = psum.tile([P, 512], F32, tag="big")
                        ps1 = psum.tile([P, 512], F32, tag="big")
                        ps2 = psum.tile([P, 512], F32, tag="big")
                        for jb in range(ib + 1):
                            off = ib * P - et_i0[jb]
                            lhsT = et_tiles[jb][:, off:off + P]
                            first = jb == 0
                            last = jb == ib
                            nc.tensor.matmul(
                                ps0[:, 0:384], lhsT=lhsT,
                                rhs=vsb[:, jb, 0:384],
                                start=first, stop=last,
                            )
                            nc.tensor.matmul(
                                ps1[:, 0:384], lhsT=lhsT,
                                rhs=vsb[:, jb, 384:768],
                                start=first, stop=last,
                            )
                            nc.tensor.matmul(
                                ps2[:, 0:257], lhsT=lhsT,
                                rhs=vsb[:, jb, 768:1025],
                                start=first, stop=last,
                            )
                        rsum = rsp.tile([P, 1], F32)
                        nc.vector.reciprocal(rsum, ps2[:, 256:257])
                        for c0, w, ps in ((0, 384, ps0), (384, 384, ps1),
                                          (768, 256, ps2)):
                            ob = outp.tile([P, 384], F32, tag="ob")
                            nc.vector.tensor_scalar_mul(
                                ob[:, 0:w], ps[:, 0:w], rsum
                            )
                            nc.sync.dma_start(
                                out=out[ib * P:(ib + 1) * P, c0:c0 + w],
                                in_=ob[:, 0:w],
                            )
    return nc


_NC_CACHE = None


def _get_nc():
    global _NC_CACHE
    if _NC_CACHE is None:
        nc = bass.Bass(
            "TRN2", target_bir_lowering=False, debug=False, num_devices=1
        )
        _emit(nc)
        _split_multi_waits(nc)
        _NC_CACHE = nc
    return _NC_CACHE


def kernel(x, Wq, Wk, Wv):
    assert x.shape == (B, T, D), x.shape
    nc = _get_nc()
    Wq = np.ascontiguousarray(Wq, dtype=np.float32)
    Wk = np.ascontiguousarray(Wk, dtype=np.float32)
    Wv = np.ascontiguousarray(Wv, dtype=np.float32)
    in_maps = [
        {
            "x": np.ascontiguousarray(x[b], dtype=np.float32),
            "Wq": Wq,
            "Wk": Wk,
            "Wv": Wv,
        }
        for b in range(B)
    ]
    res = run_bass_kernel_spmd(nc, in_maps, core_ids=list(range(B)))
    out = np.stack([res.results[b]["out"] for b in range(B)], axis=0)
    kernel.last_exec_time_ns = res.exec_time_ns
    return out



# revision 48
# speedup vs baseline: 1.1920x; 1.1920x over previous
"""Single-head causal attention (B=8, T=2048, D=1024, fp32 I/O) on 8 trn2
NeuronCores, data-parallel over batch (one batch element per core).

Per-core algorithm (all matmuls bf16 with fp32 PSUM accumulation):
  xT   = transpose(cast_bf16(x))   t0-7: PE-transpose; t8-15: DRAM+xbar
  qT   = Wq^T-stationary matmuls  -> (e, t) layout
  kT   = same                                     -> (e, t) layout
  v    = xT-stationary matmuls    -> (t, e) layout
  S^T  block (j, i) = kT(:,j)-stationary @ qT     (contraction over e)
  E^T  = exp(S^T / 32)   (no max-subtraction needed: |S/32| <~ 2)
         diagonal blocks masked by an upper-triangular 0/1 multiply
  rowsum_i = ones-matmul with E^T stationary      (PSUM accumulation over j)
  out  = (E^T-stationary @ v) * (1/rowsum)        per-partition scalar scale

The softmax normalization is applied to the AV output instead of to the
weights, so no transposes of the (T, T) attention matrix are ever needed.
Score spans are aligned to the causal diagonal so no masked block is ever
computed except the triangular diagonal blocks themselves.
"""
import sys
import types

import numpy as np

import concourse.bass as bass
import concourse.mybir as mybir
import concourse.tile as tile
from concourse.bass_utils import run_bass_kernel_spmd
from concourse.masks import make_identity, make_upper_triangular

B, T, D = 8, 2048, 1024
P = 128
TB = T // P        # 16 t-blocks
DBLK = D // P      # 8 d/e-blocks
NTS = T // 512     # 4 t-spans of 512
NES = D // 512     # 2 e-spans of 512
SCALE = 1.0 / 32.0  # 1/sqrt(D)

F32 = mybir.dt.float32
BF16 = mybir.dt.bfloat16
F8 = mybir.dt.float8e4
DR = mybir.MatmulPerfMode.DoubleRow
VW = 1032  # vsb row width: 1024 v cols + ones col at 1024 + 7 pad


def _install_ntff_hook():
    """Optional: register the axon NTFF profiling hook (the agent image's
    antenv lacks axon_hooks). Lets BASS_TRACE=1 produce exec_time_ns."""
    try:
        import antenv

        if "antenv.axon_hooks" in sys.modules:
            return
        mod = types.ModuleType("antenv.axon_hooks")
        _hook = [None]
        mod.set_axon_ntff_profile_hook = lambda h: _hook.__setitem__(0, h)
        mod.get_axon_ntff_profile_hook = lambda: _hook[0]
        sys.modules["antenv.axon_hooks"] = mod
        antenv.axon_hooks = mod
        from trn_agent_boot.trn_boot import _ntff_profile_via_ctypes

        mod.set_axon_ntff_profile_hook(
            _ntff_profile_via_ctypes("/opt/axon/libaxon_pjrt.so")
        )
    except Exception:
        pass


_install_ntff_hook()


def _split_multi_waits(nc: bass.Bass):
    """Walrus on this stack fits only ONE sync-wait per instruction, but
    Tile emits several on multi-producer instructions. Hoist the extra waits
    onto single-wait NoOps placed just before, on the same engine — the
    per-engine streams are in-order, so semantics are identical."""
    n_split = 0
    for fn in nc.m.functions:
        for bb in fn.blocks:
            out = []
            changed = False
            for inst in bb.instructions:
                si = inst.sync_info
                waits = list(si.on_wait) if si is not None and si.on_wait else []
                if len(waits) > 1:
                    for w in waits[:-1]:
                        nop = mybir.InstNoOp(
                            name=nc.get_next_instruction_name(),
                            engine=inst.engine,
                            ins=[],
                            outs=[],
                            sync_info=mybir.SyncInfo(on_wait=[w], on_update=[]),
                            bass_nofuse=True,
                        )
                        out.append(nop)
                    inst.sync_info = mybir.SyncInfo(
                        on_wait=[waits[-1]],
                        on_update=list(si.on_update or []),
                    )
                    changed = True
                    n_split += 1
                out.append(inst)
            if changed:
                bb.instructions = out
    return n_split


def _emit(nc: bass.Bass):
    x = nc.dram_tensor("x", [T, D], F32, kind="ExternalInput").ap()
    Wq = nc.dram_tensor("Wq", [D, D], F32, kind="ExternalInput").ap()
    Wk = nc.dram_tensor("Wk", [D, D], F32, kind="ExternalInput").ap()
    Wv = nc.dram_tensor("Wv", [D, D], F32, kind="ExternalInput").ap()
    out = nc.dram_tensor("out", [T, D], F32, kind="ExternalOutput").ap()

    with tile.TileContext(nc) as tc:
        from contextlib import ExitStack

        with ExitStack() as ctx:
            persist = ctx.enter_context(tc.tile_pool(name="persist", bufs=1))
            psum = ctx.enter_context(tc.tile_pool(name="psum", bufs=6, space="PSUM"))

            # ---- persistent SBUF tensors (survive the whole kernel) ----
            qT = persist.tile([P, DBLK, T], F8)         # (e, t) fp8
            kT = persist.tile([P, DBLK, T], F8)         # (e, t) fp8
            vsb = persist.tile([P, TB, VW], BF16)       # (t, e) + ones col
            # E^T tiles for i-spans 0-1 (computed early, inside phase B,
            # to fill the PE bubble while ts2/ts3 xT transposes land)
            etE = persist.tile([P, 12, 512], BF16)      # (j, i) blocks
            triu = persist.tile([P, P], BF16)
            ident = persist.tile([P, P], BF16)

            # identity first: it gates the very first PE transpose; the
            # masks are not needed until the attention phase.
            make_identity(nc, ident)
            # ones column of v_ext: the AV matmul also produces rowsums.
            nc.gpsimd.memset(vsb[:, :, 1024:1025], 1.0)
            # triu[j, i] = 1 where i >= j (keep), 0 below the diagonal.
            make_upper_triangular(nc, triu, val=1.0, diag=True)

            # ============ Phase A+B: load/cast/transpose + QKV ==========
            dram = ctx.enter_context(tc.tile_pool(name="dram", bufs=1, space="DRAM"))
            with tc.tile_pool(name="qkvp", bufs=1) as qkvp, \
                 tc.tile_pool(name="staging", bufs=4) as staging:
                xT = qkvp.tile([P, DBLK, T], BF16)          # (d, t)
                # Wq/Wk/Wv share two 8KB slots: Wv reuses Wq's slot after
                # the last q matmul has read it.
                wq_bf = qkvp.tile([P, DBLK, D], BF16, tag="wbf", bufs=2)
                wk_bf = qkvp.tile([P, DBLK, D], BF16, tag="wbf", bufs=2)
                wv_bf = qkvp.tile([P, DBLK, D], BF16, tag="wbf", bufs=2)

                xbf_dram = dram.tile([T, D], BF16)

                def x_chain_pe(tb):
                    """Load x row-block tb, cast to bf16, PE-transpose the
                    8 [128,128] sub-blocks into xT (fast path for the first
                    t-span, before the xbar pipeline has warmed up)."""
                    stage_f32 = staging.tile([P, D], F32, tag="xs32", bufs=4)
                    if tb < 2:
                        # split across two DMA queues: earlier first-byte for
                        # the tiles that gate the very first PE transposes
                        nc.sync.dma_start(
                            out=stage_f32[:, 0:512],
                            in_=x[tb * P:(tb + 1) * P, 0:512],
                        )
                        nc.sync.dma_start(
                            out=stage_f32[:, 512:D],
                            in_=x[tb * P:(tb + 1) * P, 512:D],
                        )
                    else:
                        nc.sync.dma_start(
                            out=stage_f32, in_=x[tb * P:(tb + 1) * P, :]
                        )
                    stage_bf = staging.tile([P, D], BF16, tag="xsbf", bufs=3)
                    nc.vector.tensor_copy(stage_bf[:, 0:512], stage_f32[:, 0:512])
                    nc.scalar.copy(stage_bf[:, 512:D], stage_f32[:, 512:D])
                    for db in range(DBLK):
                        # transpose as a REGULAR matmul (x-block stationary,
                        # identity moving): out = x_blk.T @ I. Issues ~2.5x
                        # faster than transpose_mode and warms HAM.
                        pst = psum.tile([P, P], F32, tag="small", bufs=2)
                        nc.tensor.matmul(
                            pst,
                            lhsT=stage_bf[:, db * P:(db + 1) * P],
                            rhs=ident,
                            start=True,
                            stop=True,
                        )
                        nc.any.tensor_copy(xT[:, db, tb * P:(tb + 1) * P], pst)

                def x_chain_store(tb):
                    """Load x row-block tb, cast to bf16, store to DRAM for
                    the xbar-transposed reload (keeps the PE free)."""
                    stage_f32 = staging.tile([P, D], F32, tag="xs32", bufs=4)
                    nc.sync.dma_start(
                        out=stage_f32, in_=x[tb * P:(tb + 1) * P, :]
                    )
                    stage_bf = staging.tile([P, D], BF16, tag="xsbf", bufs=3)
                    nc.vector.tensor_copy(stage_bf, stage_f32)
                    nc.sync.dma_start(
                        out=xbf_dram[tb * P:(tb + 1) * P, :], in_=stage_bf
                    )

                def xbar_batch(ts):
                    for db in range(DBLK):
                        nc.sync.dma_start_transpose(
                            out=xT[:, db, ts * 512:(ts + 1) * 512],
                            in_=xbf_dram[ts * 512:(ts + 1) * 512,
                                         db * P:(db + 1) * P],
                        )

                wcast = [0]

                def w_chain(w_dram, w_sb, db, dma_engine=None):
                    """Casts alternate DVE/ACT so neither paces the stream."""
                    stage_f32 = staging.tile([P, D], F32, tag="ws32", bufs=3)
                    (dma_engine or nc.sync).dma_start(
                        out=stage_f32, in_=w_dram[db * P:(db + 1) * P, :]
                    )
                    if wcast[0] % 2 == 0:
                        nc.vector.tensor_copy(w_sb[:, db, :], stage_f32)
                    else:
                        nc.scalar.copy(w_sb[:, db, :], stage_f32)
                    wcast[0] += 1

                def score_exp(jb, i0, L, et_ap):
                    """S^T block row jb over i in [i0, i0+L): matmul,
                    exp (scaled), diagonal mask if the span starts on the
                    causal diagonal."""
                    ps = psum.tile([P, 512], F32, tag="big")
                    for p4 in range(DBLK // 2):
                        nc.tensor.matmul(
                            ps[:, 0:L],
                            lhsT=kT[:, 2 * p4:2 * p4 + 2, jb * P:(jb + 1) * P],
                            rhs=qT[:, 2 * p4:2 * p4 + 2, i0:i0 + L],
                            start=(p4 == 0),
                            stop=(p4 == DBLK // 2 - 1),
                            perf_mode=DR,
                        )
                    nc.scalar.activation(
                        et_ap, ps[:, 0:L],
                        mybir.ActivationFunctionType.Exp, scale=SCALE,
                    )
                    if jb * P >= i0:  # diagonal block leads this span
                        nc.vector.tensor_mul(
                            et_ap[:, 0:P], et_ap[:, 0:P], triu
                        )

                def qk_group(w_sb, dstT, ts):
                    for eb in range(DBLK):
                        ps = psum.tile([P, 512], F32, tag="big")
                        for db in range(DBLK):
                            nc.tensor.matmul(
                                ps,
                                lhsT=w_sb[:, db, eb * P:(eb + 1) * P],
                                rhs=xT[:, db, ts * 512:(ts + 1) * 512],
                                start=(db == 0),
                                stop=(db == DBLK - 1),
                            )
                        nc.any.tensor_copy(
                            dstT[:, eb, ts * 512:(ts + 1) * 512], ps
                        )

                # DMA delivery order matches PE consumption order; late x
                # tiles are staged between B groups so their DVE casts never
                # block earlier PSUM evacuations in the static engine order.
                for i in range(4):
                    x_chain_pe(i)
                    w_chain(Wq, wq_bf, 2 * i)
                    w_chain(Wq, wq_bf, 2 * i + 1)
                for tb in range(4, 8):
                    x_chain_pe(tb)
                for db in range(DBLK):
                    w_chain(Wk, wk_bf, db)
                # first group db-outer over eb-PAIRS: starts on partial Wq
                # while keeping only 2 accumulation groups interleaved
                for pair in range(4):
                    bank0 = psum.tile([P, 512], F32, tag="big", name=f"q0a{pair}")
                    bank1 = psum.tile([P, 512], F32, tag="big", name=f"q0b{pair}")
                    eb0, eb1 = 2 * pair, 2 * pair + 1
                    for db in range(DBLK):
                        nc.tensor.matmul(
                            bank0, lhsT=wq_bf[:, db, eb0 * P:(eb0 + 1) * P],
                            rhs=xT[:, db, 0:512],
                            start=(db == 0), stop=(db == DBLK - 1),
                        )
                        nc.tensor.matmul(
                            bank1, lhsT=wq_bf[:, db, eb1 * P:(eb1 + 1) * P],
                            rhs=xT[:, db, 0:512],
                            start=(db == 0), stop=(db == DBLK - 1),
                        )
                    nc.any.tensor_copy(qT[:, eb0, 0:512], bank0)
                    nc.any.tensor_copy(qT[:, eb1, 0:512], bank1)
                x_chain_store(8)
                x_chain_store(9)
                qk_group(wq_bf, qT, 1)
                x_chain_store(10)
                x_chain_store(11)
                qk_group(wk_bf, kT, 0)
                x_chain_store(12)
                x_chain_store(13)
                qk_group(wk_bf, kT, 1)
                x_chain_store(14)
                x_chain_store(15)
                xbar_batch(2)
                # Early scores for i-spans 0-1: fills the PE bubble while
                # the ts2/ts3 xbar transposes complete.
                eidx = 0
                for s in range(2):
                    for jb in range(4 * s + 4):
                        i0 = max(s * 512, jb * P)
                        L = (s + 1) * 512 - i0
                        score_exp(jb, i0, L, etE[:, eidx, 0:L])
                        eidx += 1
                xbar_batch(3)
                qk_group(wq_bf, qT, 2)
                qk_group(wk_bf, kT, 2)
                qk_group(wq_bf, qT, 3)
                qk_group(wk_bf, kT, 3)

                # v: out[t(128), e(512)] = sum_d xT[d, t]-stat @ W[d, e]
                for db in range(DBLK):
                    w_chain(Wv, wv_bf, db, dma_engine=nc.gpsimd)
                for tb in range(TB):
                    for es in range(NES):
                        ps = psum.tile([P, 512], F32, tag="big")
                        for db in range(DBLK):
                            nc.tensor.matmul(
                                ps,
                                lhsT=xT[:, db, tb * P:(tb + 1) * P],
                                rhs=wv_bf[:, db, es * 512:(es + 1) * 512],
                                start=(db == 0),
                                stop=(db == DBLK - 1),
                            )
                        nc.any.tensor_copy(vsb[:, tb, es * 512:(es + 1) * 512], ps)

            # ================= Phase C+D: attention =====================
            with tc.tile_pool(name="etp", bufs=16) as etp, \
                 tc.tile_pool(name="outp", bufs=4) as outp, \
                 tc.tile_pool(name="rsp", bufs=4) as rsp:
                eidx = 0
                for s in range(NTS):
                    # --- scores + exp for i-span s, all jb <= 4s+3 ---
                    # (spans 0-1 were already computed inside phase B; see
                    # the early-scores fill)
                    et_tiles = []
                    et_i0 = []
                    for jb in range(4 * s + 4):
                        i0 = max(s * 512, jb * P)
                        L = (s + 1) * 512 - i0
                        if s < 2:
                            et = etE[:, eidx, :]
                            eidx += 1
                        else:
                            et = etp.tile([P, 512], BF16, tag="et")
                            score_exp(jb, i0, L, et[:, 0:L])
                        et_tiles.append(et)
                        et_i0.append(i0)

                    # --- AV (incl rowsum col) for the 4 i-blocks ---
                    for ib in range(4 * s, 4 * s + 4):
                        ps0 = psum.tile([P, 512], F32, tag="big")
                        ps1 = psum.tile([P, 512], F32, tag="big")
                        ps2 = psum.tile([P, 512], F32, tag="big")
                        # The very last block runs the rowsum bank as its
                        # own pass first, so the final reciprocal + output
                        # scaling overlap the remaining AV matmuls instead
                        # of trailing the kernel.
                        tail = ib == TB - 1
                        for jb in range(ib + 1):
                            off = ib * P - et_i0[jb]
                            lhsT = et_tiles[jb][:, off:off + P]
                            first = jb == 0
                            last = jb == ib
                            if not tail:
                                nc.tensor.matmul(
                                    ps0[:, 0:384], lhsT=lhsT,
                                    rhs=vsb[:, jb, 0:384],
                                    start=first, stop=last,
                                )
                                nc.tensor.matmul(
                                    ps1[:, 0:384], lhsT=lhsT,
                                    rhs=vsb[:, jb, 384:768],
                                    start=first, stop=last,
                                )
                            nc.tensor.matmul(
                                ps2[:, 0:257], lhsT=lhsT,
                                rhs=vsb[:, jb, 768:1025],
                                start=first, stop=last,
                            )
                        rsum = rsp.tile([P, 1], F32)
                        nc.vector.reciprocal(rsum, ps2[:, 256:257])
                        if tail:
                            for jb in range(ib + 1):
                                off = ib * P - et_i0[jb]
                                lhsT = et_tiles[jb][:, off:off + P]
                                first = jb == 0
                                last = jb == ib
                                nc.tensor.matmul(
                                    ps0[:, 0:384], lhsT=lhsT,
                                    rhs=vsb[:, jb, 0:384],
                                    start=first, stop=last,
                                )
                                nc.tensor.matmul(
                                    ps1[:, 0:384], lhsT=lhsT,
                                    rhs=vsb[:, jb, 384:768],
                                    start=first, stop=last,
                                )
                        for c0, w, ps in ((768, 256, ps2), (0, 384, ps0),
                                          (384, 384, ps1)):
                            ob = outp.tile([P, 384], F32, tag="ob")
                            nc.vector.tensor_scalar_mul(
                                ob[:, 0:w], ps[:, 0:w], rsum
                            )
                            nc.sync.dma_start(
                                out=out[ib * P:(ib + 1) * P, c0:c0 + w],
                                in_=ob[:, 0:w],
                            )
    return nc


_NC_CACHE = None


def _get_nc():
    global _NC_CACHE
    if _NC_CACHE is None:
        nc = bass.Bass(
            "TRN2", target_bir_lowering=False, debug=False, num_devices=1
        )
        _emit(nc)
        _split_multi_waits(nc)
        _NC_CACHE = nc
    return _NC_CACHE


def kernel(x, Wq, Wk, Wv):
    assert x.shape == (B, T, D), x.shape
    nc = _get_nc()
    Wq = np.ascontiguousarray(Wq, dtype=np.float32)
    Wk = np.ascontiguousarray(Wk, dtype=np.float32)
    Wv = np.ascontiguousarray(Wv, dtype=np.float32)
    in_maps = [
        {
            "x": np.ascontiguousarray(x[b], dtype=np.float32),
            "Wq": Wq,
            "Wk": Wk,
            "Wv": Wv,
        }
        for b in range(B)
    ]
    res = run_bass_kernel_spmd(nc, in_maps, core_ids=list(range(B)))
    out = np.stack([res.results[b]["out"] for b in range(B)], axis=0)
    kernel.last_exec_time_ns = res.exec_time_ns
    return out

